# revision 30
# baseline (speedup 1.0000x reference)
"""Trainium2 Bass kernel for nn_DenseRelativeLoc.

Data-parallel over batch: 32 images per NeuronCore x 8 cores.

v3 gather-first design:
  * Host lays out x transposed as xT [BPC*196 rows, 768 ch] in DRAM.
  * dma_gather (DMA-engine indexed gather, gpsimd-triggered SWDGE) pulls
    the sampled rows straight from DRAM and transposes them into
    [128 ch, 6 ch-chunk, samples] SBUF tiles -- one gather per
    (4-batch group, branch). No one-hot matmuls, no z materialization.
  * Projection h1T[hid, s] = W1^T @ featsT runs on gathered features
    only; both branches accumulate into the same PSUM tile, so
    h1 = relu(psum + b1) comes out of a single activation.
  * GEMM2 / GEMM3 in bf16 as before.
"""
import sys
import types

import numpy as np

B, C, H, W_IMG = 256, 768, 14, 14
S = 196          # sample count == H*W
HID = 512
OUT = 2
N_CORES = 8
BPC = B // N_CORES      # 32 batches per core
PAIRS = BPC // 2        # 16
KC = C // 128           # 6 chunks per branch
MJ = HID // 128         # 4 HID chunks
S2 = 2 * S              # 392: pair width
GB = 4                  # batches per gather group
NGRP = BPC // GB        # 8 groups
NIDX = 896              # padded idx count per gather (GB*S=784 -> %128)
IDXW = NIDX // 16       # 56


def _install_ntff_hook():
    try:
        import antenv.axon_hooks  # noqa: F401
        return
    except ImportError:
        pass
    try:
        from trn_agent_boot.trn_boot import _ntff_profile_via_ctypes
        hook = _ntff_profile_via_ctypes("/opt/axon/libaxon_pjrt.so")
    except Exception:
        hook = None
    mod = types.ModuleType("antenv.axon_hooks")
    mod.get_axon_ntff_profile_hook = lambda: hook
    sys.modules["antenv.axon_hooks"] = mod


def _build_nc():
    from contextlib import ExitStack

    import concourse.bass as bass
    import concourse.bacc as bacc
    import concourse.mybir as mybir
    import concourse.tile as tile

    dt = mybir.dt
    f32, bf16, i32, i16 = dt.float32, dt.bfloat16, dt.int32, dt.int16
    AF = mybir.ActivationFunctionType
    ALU = mybir.AluOpType

    nc = bacc.Bacc(None, target_bir_lowering=False)

    xt_t = nc.dram_tensor("xT", [BPC * S, C], bf16, kind="ExternalInput")
    w1_t = nc.dram_tensor("W1", [128, 2 * KC, HID], bf16, kind="ExternalInput")
    w2_t = nc.dram_tensor("W2", [HID, HID], bf16, kind="ExternalInput")
    w3_t = nc.dram_tensor("W3", [HID, OUT], bf16, kind="ExternalInput")
    b1_t = nc.dram_tensor("b1", [HID], f32, kind="ExternalInput")
    b2_t = nc.dram_tensor("b2", [HID], f32, kind="ExternalInput")
    b3_t = nc.dram_tensor("b3", [OUT], f32, kind="ExternalInput")
    idx_t = nc.dram_tensor("idxg", [128, NGRP * 2 * IDXW], i16,
                           kind="ExternalInput")
    pxs_t = nc.dram_tensor("pxs", [BPC, S2], i32, kind="ExternalInput")
    pys_t = nc.dram_tensor("pys", [BPC, S2], i32, kind="ExternalInput")
    pred_t = nc.dram_tensor("predT", [OUT, BPC * S], f32, kind="ExternalOutput")
    delta_t = nc.dram_tensor("deltaxy", [BPC * S, OUT], f32,
                             kind="ExternalOutput")

    with ExitStack() as ctx:
        tc = ctx.enter_context(tile.TileContext(nc))
        wpool = ctx.enter_context(tc.tile_pool(name="w", bufs=1))
        gxpool = ctx.enter_context(tc.tile_pool(name="gx", bufs=4))
        h1pool = ctx.enter_context(tc.tile_pool(name="h1", bufs=2))
        h2pool = ctx.enter_context(tc.tile_pool(name="h2", bufs=2))
        opool = ctx.enter_context(tc.tile_pool(name="op", bufs=1))
        idxpool = ctx.enter_context(tc.tile_pool(name="idx", bufs=1))
        zps = ctx.enter_context(tc.tile_pool(name="zps", bufs=1, space="PSUM"))
        hps = ctx.enter_context(tc.tile_pool(name="hps", bufs=1, space="PSUM"))

        # ---------- small tensors ----------
        idxt = idxpool.tile([128, NGRP * 2 * IDXW], i16, name="idxt",
                            tag="idxt")
        nc.sync.dma_start(idxt[:], idx_t[:, :])

        # ---------- gathers first: keep the gpsimd/SWDGE queue clear ------
        gx_grp = {}      # G -> (gxa, gxb) [128, KC, NIDX] bf16

        def emit_gathers(G):
            if G in gx_grp or G >= NGRP:
                return
            ga = gxpool.tile([128, KC, NIDX], bf16, name=f"gxa{G}", tag="gxa")
            gb = gxpool.tile([128, KC, NIDX], bf16, name=f"gxb{G}", tag="gxb")
            for g_out, br in ((ga, 0), (gb, 1)):
                nc.gpsimd.dma_gather(
                    g_out[:], xt_t[:, :],
                    idxt[:, (G * 2 + br) * IDXW:(G * 2 + br + 1) * IDXW],
                    num_idxs=NIDX, num_idxs_reg=NIDX, elem_size=C,
                    transpose=True,
                )
            gx_grp[G] = (ga, gb)

        emit_gathers(0)
        emit_gathers(1)
        emit_gathers(2)
        emit_gathers(3)

        pxs_sb = idxpool.tile([BPC, S2], i32, name="pxs_sb", tag="pxs_sb")
        nc.sync.dma_start(pxs_sb[:], pxs_t[:, :])
        pys_sb = idxpool.tile([BPC, S2], i32, name="pys_sb", tag="pys_sb")
        nc.sync.dma_start(pys_sb[:], pys_t[:, :])

        ones_row = wpool.tile([1, 128], bf16, name="ones_row", tag="ones_row")
        nc.vector.memset(ones_row[:], 1.0)

        # PE warm-up: fill the first-gather latency window and fully ramp
        # the p-state before the first projection matmuls
        wmt = hps.tile([128, 128], f32, name="warm", tag="warm")
        for _ in range(128):
            nc.tensor.matmul(wmt[:], ones_row[:], ones_row[:],
                             start=True, stop=True)

        # ---------- weights ----------
        w1sb = wpool.tile([128, 2 * KC, HID], bf16, name="w1sb", tag="w1sb")
        nc.sync.dma_start(w1sb[:], w1_t[:, :, :])
        w2b = []
        for k in range(MJ):
            wb = wpool.tile([128, HID], bf16, name=f"w2b{k}", tag=f"w2b{k}")
            nc.sync.dma_start(wb[:], w2_t[k * 128:(k + 1) * 128, :])
            w2b.append(wb)
        w3b = []
        for k in range(MJ):
            wb = wpool.tile([128, OUT], bf16, name=f"w3b{k}", tag=f"w3b{k}")
            nc.sync.dma_start(wb[:], w3_t[k * 128:(k + 1) * 128, :])
            w3b.append(wb)
        b1c, b2c = [], []
        for j in range(MJ):
            t1 = wpool.tile([128, 1], f32, name=f"b1c{j}", tag=f"b1c{j}")
            nc.sync.dma_start(t1[:], b1_t[j * 128:(j + 1) * 128])
            b1c.append(t1)
            t2 = wpool.tile([128, 1], f32, name=f"b2c{j}", tag=f"b2c{j}")
            nc.sync.dma_start(t2[:], b2_t[j * 128:(j + 1) * 128])
            b2c.append(t2)
        b3c = wpool.tile([OUT, 1], f32, name="b3c", tag="b3c")
        nc.sync.dma_start(b3c[:], b3_t[:])

        # ---------- deltaxy on vector (gpsimd stays free for gathers) -----
        dsub = idxpool.tile([BPC, S2], i32, name="dsub", tag="dsub")
        nc.vector.tensor_tensor(dsub[:], pxs_sb[:], pys_sb[:], ALU.subtract)
        ddel = idxpool.tile([BPC, S2], f32, name="ddel", tag="ddel")
        nc.vector.tensor_scalar(ddel[:], dsub[:], float(H - 1), None,
                                op0=ALU.add)
        nc.sync.dma_start(bass.AP(delta_t, 0, [[S2, BPC], [1, S2]]), ddel[:])

        pred_all = opool.tile([OUT, BPC * S], f32, name="pred_all",
                              tag="pred_all")

        # ---------- per-pair compute ----------
        h1_pair = {}     # P -> [128, MJ, S2] bf16

        def emit_proj(P):
            G, q = divmod(P, 2)
            ga, gb = gx_grp[G]
            h1 = h1pool.tile([128, MJ, S2], bf16, name=f"h1_{P}", tag="h1")
            h1_pair[P] = h1
            zts = [zps.tile([128, S2], f32, name=f"zt{j}_{P}", tag=f"zt{j}")
                   for j in range(MJ)]
            # all branch-a matmuls first across j, so the first pair can
            # start as soon as the a-gather lands (b still in flight)
            for half, gx in ((0, ga), (1, gb)):
                for j in range(MJ):
                    for k in range(KC):
                        nc.tensor.matmul(
                            zts[j][:],
                            w1sb[:, half * KC + k, j * 128:(j + 1) * 128],
                            gx[:, k, q * S2:(q + 1) * S2],
                            start=(half == 0 and k == 0),
                            stop=(half == 1 and k == KC - 1),
                        )
                    if half == 1:
                        if j < 2:
                            nc.scalar.activation(h1[:, j, :], zts[j][:],
                                                 AF.Relu, bias=b1c[j][:])
                        else:
                            nc.vector.tensor_scalar(h1[:, j, :], zts[j][:],
                                                    b1c[j][:], 0.0,
                                                    op0=ALU.add, op1=ALU.max)

        def emit_tail(P):
            h1 = h1_pair.pop(P)
            h2 = h2pool.tile([128, MJ, S2], bf16, name=f"h2_{P}", tag="h2")
            for j in range(MJ):
                hp = hps.tile([128, S2], f32, name=f"h2ps{j}_{P}",
                              tag=f"hps{j % 2}")
                for k in range(MJ):
                    nc.tensor.matmul(
                        hp[:],
                        w2b[k][:, j * 128:(j + 1) * 128],
                        h1[:, k, :],
                        start=(k == 0), stop=(k == MJ - 1),
                    )
                if j < 2:
                    nc.scalar.activation(h2[:, j, :], hp[:], AF.Relu,
                                         bias=b2c[j][:])
                else:
                    nc.vector.tensor_scalar(h2[:, j, :], hp[:], b2c[j][:],
                                            0.0, op0=ALU.add, op1=ALU.max)
            pp = hps.tile([OUT, S2], f32, name=f"pps_{P}", tag="pps")
            for k in range(MJ):
                nc.tensor.matmul(pp[:], w3b[k][:], h2[:, k, :],
                                 start=(k == 0), stop=(k == MJ - 1))
            nc.vector.tensor_scalar(
                pred_all[:, P * S2:(P + 1) * S2], pp[:], b3c[:], None,
                op0=ALU.add,
            )
            if P % 2 == 1:
                qd = P // 2
                nc.sync.dma_start(
                    pred_t[:, qd * 2 * S2:(qd + 1) * 2 * S2],
                    pred_all[:, qd * 2 * S2:(qd + 1) * 2 * S2],
                )

        # ---------- main loop ----------
        for P in range(PAIRS):
            if P % 2 == 0:
                emit_gathers(P // 2 + 4)
            if P % 2 == 1:
                gx_grp.pop(P // 2 - 1, None)
            emit_proj(P)
            if P >= 1:
                emit_tail(P - 1)
        emit_tail(PAIRS - 1)

    nc.finalize()
    return nc


_NC = None


def _get_nc():
    global _NC
    if _NC is None:
        _install_ntff_hook()
        _NC = _build_nc()
    return _NC


def _make_in_maps(inputs):
    import ml_dtypes
    bf16 = ml_dtypes.bfloat16

    x = np.asarray(inputs["x"], dtype=np.float32).reshape(B, C, H * W_IMG)
    x = np.asarray(x, dtype=bf16)

    W1 = np.asarray(np.asarray(inputs["W1"], dtype=np.float32), dtype=bf16)
    # [2C, HID] -> [128, 12, HID]
    w1p = np.ascontiguousarray(
        W1.reshape(2, KC, 128, HID).transpose(2, 0, 1, 3)
    ).reshape(128, 2 * KC, HID)

    W2 = np.asarray(np.asarray(inputs["W2"], dtype=np.float32), dtype=bf16)
    W3 = np.asarray(np.asarray(inputs["W3"], dtype=np.float32), dtype=bf16)
    b1 = np.asarray(inputs["b1"], dtype=np.float32)
    b2 = np.asarray(inputs["b2"], dtype=np.float32)
    b3 = np.asarray(inputs["b3"], dtype=np.float32)
    pxs = np.asarray(inputs["pxs"], dtype=np.int32)
    pys = np.asarray(inputs["pys"], dtype=np.int32)
    idx_x = pxs[:, :, 0] * W_IMG + pxs[:, :, 1]     # [B, S]
    idx_y = pys[:, :, 0] * W_IMG + pys[:, :, 1]

    in_maps = []
    for c in range(N_CORES):
        sl = slice(c * BPC, (c + 1) * BPC)
        # xT [BPC*S, C]
        xT = np.ascontiguousarray(
            x[sl].transpose(0, 2, 1).reshape(BPC * S, C))

        # global row indices per (group, branch), wrapped for dge
        base = (np.arange(BPC, dtype=np.int32) * S)[:, None]   # [BPC, 1]
        gidx = np.zeros((NGRP, 2, NIDX), dtype=np.int16)
        for br, idx in ((0, idx_x[sl]), (1, idx_y[sl])):
            gl = (idx + base).astype(np.int16).reshape(NGRP, GB * S)
            gidx[:, br, :GB * S] = gl
        idxw = gidx.reshape(NGRP, 2, IDXW, 16).transpose(3, 0, 1, 2)
        idxw = np.tile(idxw.reshape(1, 16, NGRP, 2, IDXW), (8, 1, 1, 1, 1))
        idxw = np.ascontiguousarray(idxw).reshape(128, NGRP * 2 * IDXW)

        in_maps.append({
            "xT": xT,
            "W1": w1p, "W2": W2, "W3": W3,
            "b1": b1, "b2": b2, "b3": b3,
            "idxg": idxw,
            "pxs": np.ascontiguousarray(pxs[sl].reshape(BPC, S2)),
            "pys": np.ascontiguousarray(pys[sl].reshape(BPC, S2)),
        })
    return in_maps


def _run(inputs, trace=False):
    from concourse.bass_utils import run_bass_kernel_spmd

    nc = _get_nc()
    in_maps = _make_in_maps(inputs)
    res = run_bass_kernel_spmd(
        nc, in_maps, core_ids=list(range(N_CORES)), trace=trace
    )
    pred = np.concatenate(
        [np.ascontiguousarray(res.results[c]["predT"].T) for c in range(N_CORES)],
        axis=0,
    )
    delta = np.concatenate(
        [res.results[c]["deltaxy"] for c in range(N_CORES)], axis=0
    )
    return (pred, delta), res


def kernel(**inputs):
    (pred, delta), _ = _run(inputs, trace=False)
    return pred, delta
